# revision 5
# baseline (speedup 1.0000x reference)
"""Causal self-attention (B=4, T=2048, C=768, H=12) on 8 trn2 NeuronCores.

Sharding: core = (batch b in 0..3) x (head-group g in 0..1, 6 heads each).
Each core: QKV projection for its 6 heads, causal attention, partial output
projection (its heads' rows of W_proj). Host sums the two partials per batch
and adds b_proj.

Device-side layout (per core):
  xT [768, 2048]  (host pre-transposes x[b])
  qT/kT produced as [d, t] pair tiles (lhsT = W-slice, rhs = xT)
  v produced natural [t, d] with a ones column appended per head
  S^T [k, q] = kT_block.T @ qT  -> exp on ACT -> PV: y^T += v_aug.T @ expS
    row 64 of the PV accumulator = softmax denominator (ones-column trick)
  normalize via DVE reciprocal + PE broadcast + DVE multiply
  out_partial[t, :] = sum_h yT_h.T @ Wp_h   (y^T is directly the proj lhsT)

Pipelining: QKV is emitted chunk-wise (4 chunks of 512 tokens) with the
attention for chunk c following its QKV; the S matmul for block j+1 is
emitted before the PV for block j so the PE never waits on the ACT exp;
softmax-normalization chains and the output projection for chunk c are
deferred into the QKV/attention stream of chunk c+1.

Matmul operands are stored bf16 (full PE rate, half the HBM traffic);
accumulation is fp32 in PSUM; the output partials are fp32.
"""

import sys

for _p in ("/opt/pypackages", "/opt/trn_rl_repo"):
    if _p not in sys.path:
        sys.path.insert(0, _p)

import numpy as np
import ml_dtypes

import concourse.bass as bass
import concourse.tile as tile
from concourse import bacc, mybir
from concourse.bass_utils import run_bass_kernel_spmd

B, T, C, H = 4, 2048, 768, 12
HS = C // H            # 64 head dim
HPC = 6                # heads per core
GC = HPC * HS          # 384 columns per core
NCORES = 8
NK = C // 128          # 6 contraction tiles over c_in
P = 128
F32 = mybir.dt.float32
MM = mybir.dt.bfloat16   # matmul operand dtype
NP_MM = ml_dtypes.bfloat16

NQCH = T // 512        # 4 q-chunks of 512
NTB = T // P           # 16 token blocks of 128
VPB = 3 * HS           # 192 cols per head-pair block of v


def _build_nc():
    nc = bacc.Bacc("TRN2")

    xT = nc.declare_dram_parameter("xT", [C, T], MM, isOutput=False)
    wq = nc.declare_dram_parameter("wq", [C, GC], MM, isOutput=False)
    wk = nc.declare_dram_parameter("wk", [C, GC], MM, isOutput=False)
    wv = nc.declare_dram_parameter("wv", [C, GC], MM, isOutput=False)
    wp = nc.declare_dram_parameter("wp", [GC, C], MM, isOutput=False)
    bqk = nc.declare_dram_parameter("bqk", [P, 6], F32, isOutput=False)
    bv = nc.declare_dram_parameter("bv", [1, GC], F32, isOutput=False)
    mask = nc.declare_dram_parameter("mask", [P, 2 * P], MM, isOutput=False)
    out = nc.declare_dram_parameter("out", [T, C], F32, isOutput=True)

    xTv = xT.ap().rearrange("(k p) t -> k p t", p=P)
    wqv = wq.ap().rearrange("(k p) d -> k p d", p=P)
    wkv = wk.ap().rearrange("(k p) d -> k p d", p=P)
    wvv = wv.ap().rearrange("(k p) d -> k p d", p=P)
    wpv = wp.ap().rearrange("(h p) n -> h p n", p=P)
    outv = out.ap().rearrange("(b p) n -> b p n", p=P)

    with tile.TileContext(nc) as tc:
        from contextlib import ExitStack

        with ExitStack() as ctx:
            pers = ctx.enter_context(tc.tile_pool(name="pers", bufs=1))
            # PSUM: psMM 2 x [128,1024] (2 banks each) + psY 4 x 1 bank = 8 banks
            psMM = ctx.enter_context(tc.tile_pool(name="psMM", bufs=2, space="PSUM"))
            psY = ctx.enter_context(tc.tile_pool(name="psY", bufs=4, space="PSUM"))
            work = ctx.enter_context(tc.tile_pool(name="work", bufs=3))
            ytp = ctx.enter_context(tc.tile_pool(name="ytp", bufs=2))

            # ---- persistent tiles ----
            # v layout per head-PAIR block of 192 cols: [v_even(64) | ones(1) |
            # zeros(63) | v_odd(64)].  lhsT_even = cols[0:66] -> y at rows 0-63,
            # sums at row 64, zeros at 65; lhsT_odd = cols[64:192] -> sums at
            # row 0, y at rows 64-127.
            qkT = [pers.tile([P, T], MM, name=f"qkT{i}") for i in range(6)]
            vsb = [pers.tile([P, 3 * VPB], MM, name=f"v{tb}") for tb in range(NTB)]
            wph = [pers.tile([P, C], MM, name=f"wp{hp}") for hp in range(3)]
            xt = [pers.tile([P, T], MM, name=f"xt{k}") for k in range(NK)]
            wqt = [pers.tile([P, GC], MM, name=f"wq{k}") for k in range(NK)]
            wkt = [pers.tile([P, GC], MM, name=f"wk{k}") for k in range(NK)]
            wvt = [pers.tile([P, GC], MM, name=f"wv{k}") for k in range(NK)]
            mask_sb = pers.tile([P, 2 * P], MM, name="mask")
            bqk_sb = pers.tile([P, 6], F32, name="bqk")
            bv_sb = pers.tile([1, GC], F32, name="bv")
            bvb = pers.tile([P, GC], F32, name="bvb")
            # ones rows for PE-based partition broadcasts (K=1 matmuls)
            ones128 = pers.tile([1, P], MM, name="ones128")
            bv_bf = pers.tile([1, GC], MM, name="bv_bf")

            # ---- DMA issue order: first what chunk-0 QKV needs ----
            nc.sync.dma_start(mask_sb[:], mask.ap())
            nc.sync.dma_start(bqk_sb[:], bqk.ap())
            nc.sync.dma_start(bv_sb[:], bv.ap())
            for k in range(NK):
                nc.sync.dma_start(wqt[k][:], wqv[k])
            for k in range(NK):
                nc.sync.dma_start(xt[k][:, 0:512], xTv[k][:, 0:512])
            for k in range(NK):
                nc.sync.dma_start(wkt[k][:], wkv[k])
            for k in range(NK):
                nc.sync.dma_start(wvt[k][:], wvv[k])
            for c in range(1, NQCH):
                for k in range(NK):
                    nc.sync.dma_start(
                        xt[k][:, 512 * c:512 * (c + 1)],
                        xTv[k][:, 512 * c:512 * (c + 1)],
                    )
            for hp in range(3):
                nc.sync.dma_start(wph[hp][:], wpv[hp])

            nc.vector.memset(ones128[:], 1.0)
            # bvb[p, :] = bv for all p  (PE broadcast through PSUM); stage bv
            # through a DVE copy so the matmul's producers share one sem
            with nc.allow_low_precision(reason="bias broadcast staging in bf16"):
                nc.vector.tensor_copy(out=bv_bf[:], in_=bv_sb[:])
            ps_b = psMM.tile([P, 1024], F32, tag="mm", name="ps_b")
            nc.tensor.matmul(ps_b[:, 0:GC], ones128[:], bv_bf[:],
                             start=True, stop=True)
            nc.vector.tensor_copy(out=bvb[:], in_=ps_b[:, 0:GC])

            def emit_qkv_group(c, i):
                # qT / kT pair tiles: i in 0..2 -> q pair i; 3..5 -> k pair i-3
                wt = wqt if i < 3 else wkt
                p = i % 3
                ps = psMM.tile([P, 1024], F32, tag="mm", name="ps_qk")
                for k in range(NK):
                    nc.tensor.matmul(
                        ps[:, 0:512],
                        wt[k][:, P * p:P * (p + 1)],
                        xt[k][:, 512 * c:512 * (c + 1)],
                        start=(k == 0),
                        stop=(k == NK - 1),
                    )
                # bias-add eviction on the ACT engine (idle during QKV)
                with nc.allow_low_precision(reason="qkT stored bf16"):
                    nc.scalar.add(
                        qkT[i][:, 512 * c:512 * (c + 1)],
                        ps[:, 0:512],
                        bqk_sb[:, i:i + 1],
                    )

            def emit_v_block(tb):
                # v natural [t, d] + bias, packed into pair blocks
                ps = psMM.tile([P, 1024], F32, tag="mm", name="ps_v")
                for k in range(NK):
                    nc.tensor.matmul(
                        ps[:, 0:GC],
                        xt[k][:, P * tb:P * (tb + 1)],
                        wvt[k][:],
                        start=(k == 0),
                        stop=(k == NK - 1),
                    )
                v3 = vsb[tb].rearrange("p (b e) -> p b e", e=VPB)
                ps4 = ps[:, 0:GC].rearrange("p (b o d) -> p b o d", o=2, d=HS)
                bv4 = bvb.rearrange("p (b o d) -> p b o d", o=2, d=HS)
                nc.vector.tensor_add(
                    out=v3[:, :, 0:HS],
                    in0=ps4[:, :, 0, :], in1=bv4[:, :, 0, :],
                )
                nc.vector.tensor_add(
                    out=v3[:, :, 2 * HS:3 * HS],
                    in0=ps4[:, :, 1, :], in1=bv4[:, :, 1, :],
                )
                nc.vector.memset(v3[:, :, HS:HS + 1], 1.0)
                nc.vector.memset(v3[:, :, HS + 1:2 * HS], 0.0)

            def emit_norm_pair(hp, ypsA, ypsB):
                # Even head: sums at ypsA row 64 -> stage to SBUF (DVE,
                # partition-aligned), DMA to partition 0 (gpsimd custom ops
                # need base-0 operands on HW), reciprocal at base 0, gpsimd
                # partition_broadcast to rows 0-63.  Odd head: sums at ypsB
                # row 0 -> reciprocal at base 0, gpsimd-broadcast across all
                # 128 rows (base-0 dst), multiply lane-aligned at base 64.
                st = work.tile([HS + 1, 512], F32, tag="st", name="st")
                nc.vector.tensor_copy(out=st[HS:HS + 1, :],
                                      in_=ypsA[HS:HS + 1, :])
                stE = work.tile([1, 512], F32, tag="stE", name="stE")
                nc.sync.dma_start(stE[:], st[HS:HS + 1, :])
                rtE = work.tile([1, 512], F32, tag="rtE", name="rtE")
                nc.vector.reciprocal_approx_fast(out=rtE[:], in_=stE[:])
                rbiA = work.tile([HS, 512], F32, tag="rbiA", name="rbiA")
                nc.gpsimd.partition_broadcast(rbiA[:], rtE[:], channels=HS)

                stB = work.tile([1, 512], F32, tag="stB", name="stB")
                nc.vector.tensor_copy(out=stB[:], in_=ypsB[0:1, :])
                rtB = work.tile([1, 512], F32, tag="rtB", name="rtB")
                nc.vector.reciprocal_approx_fast(out=rtB[:], in_=stB[:])
                rbiB = work.tile([P, 512], F32, tag="rbiB", name="rbiB")
                nc.gpsimd.partition_broadcast(rbiB[0:P, :], rtB[:],
                                              channels=P)

                yt = ytp.tile([P, 512], MM, tag=f"ytp{hp}", name=f"ytp{hp}")
                nc.vector.tensor_mul(out=yt[0:HS, :], in0=ypsA[0:HS, :],
                                     in1=rbiA[:])
                nc.vector.tensor_mul(out=yt[HS:P, :], in0=ypsB[HS:P, :],
                                     in1=rbiB[HS:P, :])
                return yt

            def emit_attn_chunk(c, pend, ytiles):
                # pend/ytiles: the previous pair's un-normalized PSUM output;
                # its norm chain is emitted under the next pair's first block
                # (or by the caller across chunk boundaries).
                jlast = 4 * c + 3
                for hp in range(3):
                    qTA = qkT[hp][0:HS, :]
                    qTB = qkT[hp][HS:P, :]
                    kTA = qkT[3 + hp][0:HS, :]
                    kTB = qkT[3 + hp][HS:P, :]
                    vp = [
                        vsb[j].rearrange("p (b e) -> p b e", e=VPB)[:, hp, :]
                        for j in range(jlast + 1)
                    ]

                    ypsA = psY.tile([HS + 2, 512], F32, tag="y", name="ypsA")
                    ypsB = psY.tile([P, 512], F32, tag="y", name="ypsB")
                    prev = None
                    for j in range(jlast + 1):
                        m = j - 4 * c
                        qs = P * m if m > 0 else 0
                        sps = psMM.tile([P, 1024], F32, tag="mm", name="ps_s")
                        es = work.tile([P, 1024], MM, tag="es", name="es", bufs=4)
                        # both heads' S blocks, row-tiled (A rows 0-63, B 64-127)
                        nc.tensor.matmul(
                            sps[:, qs:512],
                            kTA[:, P * j:P * (j + 1)],
                            qTA[:, 512 * c + qs:512 * (c + 1)],
                            start=True, stop=True,
                        )
                        nc.tensor.matmul(
                            sps[:, 512 + qs:1024],
                            kTB[:, P * j:P * (j + 1)],
                            qTB[:, 512 * c + qs:512 * (c + 1)],
                            start=True, stop=True,
                        )
                        if qs > 0:
                            # one 3D-AP exp over both heads' [qs:512] halves
                            es2 = es.rearrange("p (u n) -> p u n", n=512)
                            sp2 = sps.rearrange("p (u n) -> p u n", n=512)
                            nc.scalar.activation(
                                out=es2[:, :, qs:512], in_=sp2[:, :, qs:512],
                                func=mybir.ActivationFunctionType.Exp,
                                scale=1.0 / 8.0)
                        else:
                            nc.scalar.activation(
                                out=es[:], in_=sps[:],
                                func=mybir.ActivationFunctionType.Exp,
                                scale=1.0 / 8.0)
                        if m >= 0:
                            # one double-wide masked multiply over both heads'
                            # diagonal sub-blocks (mask_sb is [128, 256])
                            es2 = es.rearrange("p (u n) -> p u n", n=512)
                            mk2 = mask_sb.rearrange("p (u n) -> p u n", n=P)
                            nc.vector.tensor_mul(
                                out=es2[:, :, qs:qs + P],
                                in0=es2[:, :, qs:qs + P], in1=mk2[:])
                        if prev is not None:
                            pj, pqs, pes = prev
                            nc.tensor.matmul(
                                ypsA[:, pqs:512], vp[pj][:, 0:HS + 2],
                                pes[:, pqs:512],
                                start=(pj == 0), stop=(pj == jlast),
                            )
                            nc.tensor.matmul(
                                ypsB[:, pqs:512], vp[pj][:, HS:VPB],
                                pes[:, 512 + pqs:1024],
                                start=(pj == 0), stop=(pj == jlast),
                            )
                        elif pend is not None:
                            # emit the previous pair's norm chain under this
                            # pair's first block so it overlaps S/exp work
                            php, pA, pB = pend
                            ytiles[php] = emit_norm_pair(php, pA, pB)
                            pend = None
                        prev = (j, qs, es)
                    pj, pqs, pes = prev
                    nc.tensor.matmul(
                        ypsA[:, pqs:512], vp[pj][:, 0:HS + 2],
                        pes[:, pqs:512],
                        start=(pj == 0), stop=(pj == jlast),
                    )
                    nc.tensor.matmul(
                        ypsB[:, pqs:512], vp[pj][:, HS:VPB],
                        pes[:, 512 + pqs:1024],
                        start=(pj == 0), stop=(pj == jlast),
                    )
                    pend = (hp, ypsA, ypsB)
                return pend

            def emit_proj(c, ytiles):
                # projection for the 4 token-blocks of this chunk (K=128)
                for tq in range(4):
                    tb = 4 * c + tq
                    pps = psMM.tile([P, 1024], F32, tag="mm", name="ps_o")
                    for hp in range(3):
                        for n0, nn in ((0, 512), (512, 256)):
                            nc.tensor.matmul(
                                pps[:, n0:n0 + nn],
                                ytiles[hp][:, P * tq:P * (tq + 1)],
                                wph[hp][:, n0:n0 + nn],
                                start=(hp == 0),
                                stop=(hp == 2),
                            )
                    ot = work.tile([P, C], F32, tag="ot", name="ot")
                    nc.scalar.copy(out=ot[:], in_=pps[:, 0:C])
                    nc.sync.dma_start(outv[tb], ot[:])

            # ---- main schedule: QKV(c) ... attn(c) with norm/proj of c-1
            # deferred into chunk c's stream ----
            pend = None
            ytiles_prev = None
            for c in range(NQCH):
                ytiles = [None] * 3
                emit_qkv_group(c, 0)
                if pend is not None:
                    # previous chunk's last pair: norm under QKV of this chunk
                    php, pA, pB = pend
                    ytiles_prev[php] = emit_norm_pair(php, pA, pB)
                    pend = None
                for i in range(1, 6):
                    emit_qkv_group(c, i)
                for tq in range(4):
                    emit_v_block(4 * c + tq)
                if ytiles_prev is not None:
                    emit_proj(c - 1, ytiles_prev)
                pend = emit_attn_chunk(c, pend, ytiles)
                ytiles_prev = ytiles

            php, pA, pB = pend
            ytiles_prev[php] = emit_norm_pair(php, pA, pB)
            emit_proj(NQCH - 1, ytiles_prev)

    nc.compile()
    return nc


_nc_cache = None
last_results = None


def _get_nc():
    global _nc_cache
    if _nc_cache is None:
        _nc_cache = _build_nc()
    return _nc_cache


def make_in_maps(x, W_attn, b_attn, W_proj):
    x = np.asarray(x, np.float32)
    W_attn = np.asarray(W_attn, np.float32)
    b_attn = np.asarray(b_attn, np.float32)
    W_proj = np.asarray(W_proj, np.float32)

    kk, qq = np.meshgrid(np.arange(P), np.arange(P), indexing="ij")
    mask = np.tile((qq >= kk).astype(NP_MM), (1, 2))

    in_maps = []
    for core in range(NCORES):
        b, g = divmod(core, 2)
        hs = slice(GC * g, GC * (g + 1))
        bq = b_attn[0:C][hs]
        bk = b_attn[C:2 * C][hs]
        bvs = b_attn[2 * C:3 * C][hs]
        bqk = np.stack(
            [bq[P * p:P * (p + 1)] for p in range(3)]
            + [bk[P * p:P * (p + 1)] for p in range(3)],
            axis=1,
        ).astype(np.float32)
        in_maps.append({
            "xT": np.ascontiguousarray(x[b].T).astype(NP_MM),
            "wq": np.ascontiguousarray(W_attn[:, 0:C][:, hs]).astype(NP_MM),
            "wk": np.ascontiguousarray(W_attn[:, C:2 * C][:, hs]).astype(NP_MM),
            "wv": np.ascontiguousarray(W_attn[:, 2 * C:3 * C][:, hs]).astype(NP_MM),
            "wp": np.ascontiguousarray(W_proj[hs, :]).astype(NP_MM),
            "bqk": np.ascontiguousarray(bqk),
            "bv": np.ascontiguousarray(bvs.reshape(1, GC)),
            "mask": mask,
        })
    return in_maps


def kernel(x, W_attn, b_attn, W_proj, b_proj, _trace=False):
    global last_results
    nc = _get_nc()
    in_maps = make_in_maps(x, W_attn, b_attn, W_proj)
    res = run_bass_kernel_spmd(nc, in_maps, list(range(NCORES)), trace=_trace)
    last_results = res
    out = np.zeros((B, T, C), np.float32)
    for core in range(NCORES):
        out[core // 2] += res.results[core]["out"]
    out += np.asarray(b_proj, np.float32)[None, None, :]
    return out


# revision 7
# speedup vs baseline: 1.1558x; 1.1558x over previous
"""Causal self-attention (B=4, T=2048, C=768, H=12) on 8 trn2 NeuronCores.

Sharding: core = (batch b in 0..3) x (head-group g in 0..1, 6 heads each).
Each core: QKV projection for its 6 heads, causal attention, partial output
projection (its heads' rows of W_proj). Host sums the two partials per batch
and adds b_proj.

Device-side layout (per core):
  xT [768, 2048]  (host pre-transposes x[b])
  qT/kT produced as [d, t] pair tiles (lhsT = W-slice, rhs = xT)
  v produced natural [t, d] with a ones column appended per head
  S^T [k, q] = kT_block.T @ qT  -> exp on ACT -> PV: y^T += v_aug.T @ expS
    row 64 of the PV accumulator = softmax denominator (ones-column trick)
  normalize via DVE reciprocal + PE broadcast + DVE multiply
  out_partial[t, :] = sum_h yT_h.T @ Wp_h   (y^T is directly the proj lhsT)

Pipelining: QKV is emitted chunk-wise (4 chunks of 512 tokens) with the
attention for chunk c following its QKV; the S matmul for block j+1 is
emitted before the PV for block j so the PE never waits on the ACT exp;
softmax-normalization chains and the output projection for chunk c are
deferred into the QKV/attention stream of chunk c+1.

Matmul operands are stored bf16 (full PE rate, half the HBM traffic);
accumulation is fp32 in PSUM; the output partials are fp32.
"""

import sys

for _p in ("/opt/pypackages", "/opt/trn_rl_repo"):
    if _p not in sys.path:
        sys.path.insert(0, _p)

import numpy as np
import ml_dtypes

import concourse.bass as bass
import concourse.tile as tile
from concourse import bacc, mybir
from concourse.bass_utils import run_bass_kernel_spmd

B, T, C, H = 4, 2048, 768, 12
HS = C // H            # 64 head dim
HPC = 6                # heads per core
GC = HPC * HS          # 384 columns per core
NCORES = 8
NK = C // 128          # 6 contraction tiles over c_in
P = 128
F32 = mybir.dt.float32
MM = mybir.dt.bfloat16   # matmul operand dtype
NP_MM = ml_dtypes.bfloat16

NQCH = T // 512        # 4 q-chunks of 512
NTB = T // P           # 16 token blocks of 128
VPB = 3 * HS           # 192 cols per head-pair block of v


def _build_nc():
    nc = bacc.Bacc("TRN2")

    xT = nc.declare_dram_parameter("xT", [C, T], MM, isOutput=False)
    wq = nc.declare_dram_parameter("wq", [C, GC], MM, isOutput=False)
    wk = nc.declare_dram_parameter("wk", [C, GC], MM, isOutput=False)
    wv = nc.declare_dram_parameter("wv", [C, GC], MM, isOutput=False)
    wp = nc.declare_dram_parameter("wp", [GC, C], MM, isOutput=False)
    bqk = nc.declare_dram_parameter("bqk", [P, 6], F32, isOutput=False)
    bv = nc.declare_dram_parameter("bv", [1, GC], F32, isOutput=False)
    mask = nc.declare_dram_parameter("mask", [P, 2 * P], MM, isOutput=False)
    out = nc.declare_dram_parameter("out", [T, C], F32, isOutput=True)

    xTv = xT.ap().rearrange("(k p) t -> p k t", p=P)
    wqv = wq.ap().rearrange("(k p) d -> p k d", p=P)
    wkv = wk.ap().rearrange("(k p) d -> p k d", p=P)
    wvv = wv.ap().rearrange("(k p) d -> p k d", p=P)
    wpv = wp.ap().rearrange("(h p) n -> h p n", p=P)
    outv = out.ap().rearrange("(b p) n -> b p n", p=P)

    with tile.TileContext(nc) as tc:
        from contextlib import ExitStack

        with ExitStack() as ctx:
            pers = ctx.enter_context(tc.tile_pool(name="pers", bufs=1))
            # PSUM: psMM 2 x [128,1024] (2 banks each) + psY 4 x 1 bank = 8 banks
            psMM = ctx.enter_context(tc.tile_pool(name="psMM", bufs=2, space="PSUM"))
            psY = ctx.enter_context(tc.tile_pool(name="psY", bufs=4, space="PSUM"))
            work = ctx.enter_context(tc.tile_pool(name="work", bufs=3))
            ytp = ctx.enter_context(tc.tile_pool(name="ytp", bufs=2))

            # ---- persistent tiles ----
            # v layout per head-PAIR block of 192 cols: [v_even(64) | ones(1) |
            # zeros(63) | v_odd(64)].  lhsT_even = cols[0:66] -> y at rows 0-63,
            # sums at row 64, zeros at 65; lhsT_odd = cols[64:192] -> sums at
            # row 0, y at rows 64-127.
            qkT = [pers.tile([P, T], MM, name=f"qkT{i}") for i in range(6)]
            vsb = [pers.tile([P, 3 * VPB], MM, name=f"v{tb}") for tb in range(NTB)]
            wph = [pers.tile([P, C], MM, name=f"wp{hp}") for hp in range(3)]
            xt_all = pers.tile([P, NK, T], MM, name="xt_all")
            wqt_all = pers.tile([P, NK, GC], MM, name="wqt_all")
            wkt_all = pers.tile([P, NK, GC], MM, name="wkt_all")
            wvt_all = pers.tile([P, NK, GC], MM, name="wvt_all")
            xt = [xt_all[:, k, :] for k in range(NK)]
            wqt = [wqt_all[:, k, :] for k in range(NK)]
            wkt = [wkt_all[:, k, :] for k in range(NK)]
            wvt = [wvt_all[:, k, :] for k in range(NK)]
            mask_sb = pers.tile([P, 2 * P], MM, name="mask")
            bqk_sb = pers.tile([P, 6], F32, name="bqk")
            bv_sb = pers.tile([1, GC], F32, name="bv")
            bvb = pers.tile([P, GC], F32, name="bvb")
            # ones rows for PE-based partition broadcasts (K=1 matmuls)
            ones128 = pers.tile([1, P], MM, name="ones128")
            bv_bf = pers.tile([1, GC], MM, name="bv_bf")

            # ---- DMA issue order: first what chunk-0 QKV needs ----
            nc.sync.dma_start(bqk_sb[:], bqk.ap())
            nc.sync.dma_start(wqt_all[:], wqv)
            nc.sync.dma_start(xt_all[:, :, 0:512], xTv[:, :, 0:512])
            nc.sync.dma_start(wkt_all[:], wkv)
            nc.sync.dma_start(wvt_all[:], wvv)
            nc.sync.dma_start(mask_sb[:], mask.ap())
            nc.sync.dma_start(bv_sb[:], bv.ap())
            for c in range(1, NQCH):
                nc.sync.dma_start(
                    xt_all[:, :, 512 * c:512 * (c + 1)],
                    xTv[:, :, 512 * c:512 * (c + 1)],
                )
            for hp in range(3):
                nc.sync.dma_start(wph[hp][:], wpv[hp])

            def emit_bvb():
                # bvb[p, :] = bv for all p (PE broadcast through PSUM); off
                # the startup critical path -- first consumer is emit_v_block
                nc.vector.memset(ones128[:], 1.0)
                with nc.allow_low_precision(reason="bias bcast staging bf16"):
                    nc.vector.tensor_copy(out=bv_bf[:], in_=bv_sb[:])
                ps_b = psMM.tile([P, 1024], F32, tag="mm", name="ps_b")
                nc.tensor.matmul(ps_b[:, 0:GC], ones128[:], bv_bf[:],
                                 start=True, stop=True)
                nc.vector.tensor_copy(out=bvb[:], in_=ps_b[:, 0:GC])

            def emit_qkv_group(c, i):
                # qT / kT pair tiles: i in 0..2 -> q pair i; 3..5 -> k pair i-3
                wt = wqt if i < 3 else wkt
                p = i % 3
                ps = psMM.tile([P, 1024], F32, tag="mm", name="ps_qk")
                for k in range(NK):
                    nc.tensor.matmul(
                        ps[:, 0:512],
                        wt[k][:, P * p:P * (p + 1)],
                        xt[k][:, 512 * c:512 * (c + 1)],
                        start=(k == 0),
                        stop=(k == NK - 1),
                    )
                # bias-add eviction on the ACT engine (idle during QKV)
                with nc.allow_low_precision(reason="qkT stored bf16"):
                    nc.scalar.add(
                        qkT[i][:, 512 * c:512 * (c + 1)],
                        ps[:, 0:512],
                        bqk_sb[:, i:i + 1],
                    )

            def emit_v_block(tb):
                # v natural [t, d] + bias, packed into pair blocks
                ps = psMM.tile([P, 1024], F32, tag="mm", name="ps_v")
                for k in range(NK):
                    nc.tensor.matmul(
                        ps[:, 0:GC],
                        xt[k][:, P * tb:P * (tb + 1)],
                        wvt[k][:],
                        start=(k == 0),
                        stop=(k == NK - 1),
                    )
                v3 = vsb[tb].rearrange("p (b e) -> p b e", e=VPB)
                ps4 = ps[:, 0:GC].rearrange("p (b o d) -> p b o d", o=2, d=HS)
                bv4 = bvb.rearrange("p (b o d) -> p b o d", o=2, d=HS)
                nc.vector.tensor_add(
                    out=v3[:, :, 0:HS],
                    in0=ps4[:, :, 0, :], in1=bv4[:, :, 0, :],
                )
                nc.vector.tensor_add(
                    out=v3[:, :, 2 * HS:3 * HS],
                    in0=ps4[:, :, 1, :], in1=bv4[:, :, 1, :],
                )
                nc.vector.memset(v3[:, :, HS:HS + 1], 1.0)
                nc.vector.memset(v3[:, :, HS + 1:2 * HS], 0.0)

            def emit_norm_pair(hp, ypsA, ypsB):
                # Even head: sums at ypsA row 64 -> stage to SBUF (DVE,
                # partition-aligned), DMA to partition 0 (gpsimd custom ops
                # need base-0 operands on HW), reciprocal at base 0, gpsimd
                # partition_broadcast to rows 0-63.  Odd head: sums at ypsB
                # row 0 -> reciprocal at base 0, gpsimd-broadcast across all
                # 128 rows (base-0 dst), multiply lane-aligned at base 64.
                st = work.tile([HS + 1, 512], F32, tag="st", name="st")
                nc.vector.tensor_copy(out=st[HS:HS + 1, :],
                                      in_=ypsA[HS:HS + 1, :])
                stE = work.tile([1, 512], F32, tag="stE", name="stE")
                nc.gpsimd.dma_start(stE[:], st[HS:HS + 1, :])
                rtE = work.tile([1, 512], F32, tag="rtE", name="rtE")
                nc.vector.reciprocal_approx_fast(out=rtE[:], in_=stE[:])
                rbiA = work.tile([HS, 512], F32, tag="rbiA", name="rbiA")
                nc.gpsimd.partition_broadcast(rbiA[:], rtE[:], channels=HS)

                stB = work.tile([1, 512], F32, tag="stB", name="stB")
                nc.vector.tensor_copy(out=stB[:], in_=ypsB[0:1, :])
                rtB = work.tile([1, 512], F32, tag="rtB", name="rtB")
                nc.vector.reciprocal_approx_fast(out=rtB[:], in_=stB[:])
                rbiB = work.tile([P, 512], F32, tag="rbiB", name="rbiB")
                nc.gpsimd.partition_broadcast(rbiB[0:P, :], rtB[:],
                                              channels=P)

                yt = ytp.tile([P, 512], MM, tag=f"ytp{hp}", name=f"ytp{hp}")
                nc.vector.tensor_mul(out=yt[0:HS, :], in0=ypsA[0:HS, :],
                                     in1=rbiA[:])
                nc.vector.tensor_mul(out=yt[HS:P, :], in0=ypsB[HS:P, :],
                                     in1=rbiB[HS:P, :])
                return yt

            def emit_attn_chunk(c, pend, ytiles):
                # pend/ytiles: the previous pair's un-normalized PSUM output;
                # its norm chain is emitted under the next pair's first block
                # (or by the caller across chunk boundaries).
                jlast = 4 * c + 3
                for hp in range(3):
                    qTA = qkT[hp][0:HS, :]
                    qTB = qkT[hp][HS:P, :]
                    kTA = qkT[3 + hp][0:HS, :]
                    kTB = qkT[3 + hp][HS:P, :]
                    vp = [
                        vsb[j].rearrange("p (b e) -> p b e", e=VPB)[:, hp, :]
                        for j in range(jlast + 1)
                    ]

                    ypsA = psY.tile([HS + 2, 512], F32, tag="y", name="ypsA")
                    ypsB = psY.tile([P, 512], F32, tag="y", name="ypsB")
                    prev = None
                    for j in range(jlast + 1):
                        m = j - 4 * c
                        qs = P * m if m > 0 else 0
                        sps = psMM.tile([P, 1024], F32, tag="mm", name="ps_s")
                        es = work.tile([P, 1024], MM, tag="es", name="es", bufs=4)
                        # both heads' S blocks, row-tiled (A rows 0-63, B 64-127)
                        nc.tensor.matmul(
                            sps[:, qs:512],
                            kTA[:, P * j:P * (j + 1)],
                            qTA[:, 512 * c + qs:512 * (c + 1)],
                            start=True, stop=True,
                        )
                        nc.tensor.matmul(
                            sps[:, 512 + qs:1024],
                            kTB[:, P * j:P * (j + 1)],
                            qTB[:, 512 * c + qs:512 * (c + 1)],
                            start=True, stop=True,
                        )
                        if qs > 0:
                            # one 3D-AP exp over both heads' [qs:512] halves
                            es2 = es.rearrange("p (u n) -> p u n", n=512)
                            sp2 = sps.rearrange("p (u n) -> p u n", n=512)
                            nc.scalar.activation(
                                out=es2[:, :, qs:512], in_=sp2[:, :, qs:512],
                                func=mybir.ActivationFunctionType.Exp,
                                scale=1.0 / 8.0)
                        else:
                            nc.scalar.activation(
                                out=es[:], in_=sps[:],
                                func=mybir.ActivationFunctionType.Exp,
                                scale=1.0 / 8.0)
                        if m >= 0:
                            # one double-wide masked multiply over both heads'
                            # diagonal sub-blocks (mask_sb is [128, 256])
                            es2 = es.rearrange("p (u n) -> p u n", n=512)
                            mk2 = mask_sb.rearrange("p (u n) -> p u n", n=P)
                            nc.vector.tensor_mul(
                                out=es2[:, :, qs:qs + P],
                                in0=es2[:, :, qs:qs + P], in1=mk2[:])
                        if prev is not None:
                            pj, pqs, pes = prev
                            nc.tensor.matmul(
                                ypsA[:, pqs:512], vp[pj][:, 0:HS + 2],
                                pes[:, pqs:512],
                                start=(pj == 0), stop=(pj == jlast),
                            )
                            nc.tensor.matmul(
                                ypsB[:, pqs:512], vp[pj][:, HS:VPB],
                                pes[:, 512 + pqs:1024],
                                start=(pj == 0), stop=(pj == jlast),
                            )
                        elif pend is not None:
                            # emit the previous pair's norm chain under this
                            # pair's first block so it overlaps S/exp work
                            php, pA, pB = pend
                            ytiles[php] = emit_norm_pair(php, pA, pB)
                            pend = None
                        prev = (j, qs, es)
                    pj, pqs, pes = prev
                    nc.tensor.matmul(
                        ypsA[:, pqs:512], vp[pj][:, 0:HS + 2],
                        pes[:, pqs:512],
                        start=(pj == 0), stop=(pj == jlast),
                    )
                    nc.tensor.matmul(
                        ypsB[:, pqs:512], vp[pj][:, HS:VPB],
                        pes[:, 512 + pqs:1024],
                        start=(pj == 0), stop=(pj == jlast),
                    )
                    pend = (hp, ypsA, ypsB)
                return pend

            def emit_proj(c, ytiles):
                # projection for the 4 token-blocks of this chunk (K=128)
                for tq in range(4):
                    tb = 4 * c + tq
                    pps = psMM.tile([P, 1024], F32, tag="mm", name="ps_o")
                    for hp in range(3):
                        for n0, nn in ((0, 512), (512, 256)):
                            nc.tensor.matmul(
                                pps[:, n0:n0 + nn],
                                ytiles[hp][:, P * tq:P * (tq + 1)],
                                wph[hp][:, n0:n0 + nn],
                                start=(hp == 0),
                                stop=(hp == 2),
                            )
                    ot = work.tile([P, C], F32, tag="ot", name="ot")
                    nc.scalar.copy(out=ot[:], in_=pps[:, 0:C])
                    nc.sync.dma_start(outv[tb], ot[:])

            # ---- main schedule: QKV(c) ... attn(c) with norm/proj of c-1
            # deferred into chunk c's stream ----
            pend = None
            ytiles_prev = None
            for c in range(NQCH):
                ytiles = [None] * 3
                emit_qkv_group(c, 0)
                if c == 0:
                    pass
                if pend is not None:
                    # previous chunk's last pair: norm under QKV of this chunk
                    php, pA, pB = pend
                    ytiles_prev[php] = emit_norm_pair(php, pA, pB)
                    pend = None
                for i in range(1, 6):
                    emit_qkv_group(c, i)
                if c == 0:
                    emit_bvb()
                for tq in range(4):
                    emit_v_block(4 * c + tq)
                if ytiles_prev is not None:
                    emit_proj(c - 1, ytiles_prev)
                pend = emit_attn_chunk(c, pend, ytiles)
                ytiles_prev = ytiles

            php, pA, pB = pend
            ytiles_prev[php] = emit_norm_pair(php, pA, pB)
            emit_proj(NQCH - 1, ytiles_prev)

    nc.compile()
    return nc


_nc_cache = None
last_results = None


def _get_nc():
    global _nc_cache
    if _nc_cache is None:
        _nc_cache = _build_nc()
    return _nc_cache


def make_in_maps(x, W_attn, b_attn, W_proj):
    x = np.asarray(x, np.float32)
    W_attn = np.asarray(W_attn, np.float32)
    b_attn = np.asarray(b_attn, np.float32)
    W_proj = np.asarray(W_proj, np.float32)

    kk, qq = np.meshgrid(np.arange(P), np.arange(P), indexing="ij")
    mask = np.tile((qq >= kk).astype(NP_MM), (1, 2))

    in_maps = []
    for core in range(NCORES):
        b, g = divmod(core, 2)
        hs = slice(GC * g, GC * (g + 1))
        bq = b_attn[0:C][hs]
        bk = b_attn[C:2 * C][hs]
        bvs = b_attn[2 * C:3 * C][hs]
        bqk = np.stack(
            [bq[P * p:P * (p + 1)] for p in range(3)]
            + [bk[P * p:P * (p + 1)] for p in range(3)],
            axis=1,
        ).astype(np.float32)
        in_maps.append({
            "xT": np.ascontiguousarray(x[b].T).astype(NP_MM),
            "wq": np.ascontiguousarray(W_attn[:, 0:C][:, hs]).astype(NP_MM),
            "wk": np.ascontiguousarray(W_attn[:, C:2 * C][:, hs]).astype(NP_MM),
            "wv": np.ascontiguousarray(W_attn[:, 2 * C:3 * C][:, hs]).astype(NP_MM),
            "wp": np.ascontiguousarray(W_proj[hs, :]).astype(NP_MM),
            "bqk": np.ascontiguousarray(bqk),
            "bv": np.ascontiguousarray(bvs.reshape(1, GC)),
            "mask": mask,
        })
    return in_maps


def kernel(x, W_attn, b_attn, W_proj, b_proj, _trace=False):
    global last_results
    nc = _get_nc()
    in_maps = make_in_maps(x, W_attn, b_attn, W_proj)
    res = run_bass_kernel_spmd(nc, in_maps, list(range(NCORES)), trace=_trace)
    last_results = res
    out = np.zeros((B, T, C), np.float32)
    for core in range(NCORES):
        out[core // 2] += res.results[core]["out"]
    out += np.asarray(b_proj, np.float32)[None, None, :]
    return out
